# revision 15
# baseline (speedup 1.0000x reference)
"""Multihead attention (B=2, L=2048, D=1024, 16 heads) on 8 trn2 cores.

Sharding: tensor-parallel over heads — 2 heads per core. Each core computes
q/k/v projections for its 128 columns of Wq/Wk/Wv, full attention for its two
heads, and a partial output projection against its 128 rows of Wo. The host
sums the 8 bf16 partials and adds bo.

Compute is bf16 on the PE with fp32 PSUM accumulation. Layouts:
  qT/kT: [128(d_local), B*L]      — contraction-major for the scoresT matmuls
  vaug:  [s, 64]+ones column      — streamed rhs for the transposed-av; the
                                    ones column emits the softmax denominator
  scoresT[s, l] per (b, l-chunk), both heads in one 2-bank PSUM tile; exp on
  ScalarE (no max subtraction: scores ~ N(0,1)).

Transposed attn@v: out[l-tile, d] accumulates with exs (the exp'd scoresT
block) as the stationary weights and vaug as the 65-col stream — full PE
column utilization (vs M=65 in the d-major layout). The denominator lands
per-PARTITION (col 64 of each l-row), so normalization folds into a
diagonal-matmul transpose: D = I * rcp (DVE per-partition scale of a host-
provided identity), then oT[hd, l] = o_sb^T @ D on the PE — transpose and
per-head normalize in one matmul, no gpsimd broadcast. bv is added at the
oT evacuation (per-partition there), exact since sum(attn)=1 post-normalize.

PSUM (8 banks): scores 2x2, av 2x1 ([128,4,65] per head, 4 sub-bank matmul
accumulation regions), aux 2x1 (q/k/v projection accumulators run
sequentially one slot at a time, o-proj halves [128,512], oT transposes).

Schedule: ScalarE-exp-paced (~1.1us per s-tile; attention PE work is
~0.7us). Batch-1 projections drip-feed as group-atomic filler chains (a
projection chunk's accumulating PSUM tile must not have other aux
allocations interleaved); the per-chunk o-path (rcp/D/oT/o-proj) closures
are self-contained singles on a priority queue that runs between groups.
Each chunk's last `lag` attn@v groups carry into the next chunk's first
periods so the exp pipeline drains under fresh scores.
"""

from collections import deque
from contextlib import ExitStack

import ml_dtypes
import numpy as np

import concourse.bacc as bacc
import concourse.mybir as mybir
import concourse.tile as tile
from concourse.bass_utils import run_bass_kernel_spmd

D_MODEL = 1024
N_HEAD = 16
HEAD_DIM = 64
B = 2
L = 2048
N_CORES = 8
HPC = N_HEAD // N_CORES  # heads per core
MLOC = HPC * HEAD_DIM  # 128: local d width per core

F32 = mybir.dt.float32
BF16 = mybir.dt.bfloat16
NPBF16 = ml_dtypes.bfloat16


def build_nc(Lb=L, lc_size=512, nch=512, upfront=None):
    """Build the per-core Bass program. Lb = sequence length per batch."""
    BLb = B * Lb
    KT = D_MODEL // 128  # 8 contraction tiles for the projections
    n_nch = BLb // nch  # projection column chunks
    st_per_nch = nch // 128  # s-tiles per projection chunk
    n_lc = Lb // lc_size  # attention l-chunks per batch
    n_st = Lb // 128  # s-tiles per batch
    n_lt = lc_size // 128  # l-tiles (128 wide) per attention chunk
    if upfront is None:
        # All but one batch-0 proj chunk dense before attention; the last
        # one drips in during the first periods (its kT/v are consumed from
        # s-tile 12 onward, ~period 12+).
        upfront = max(1, Lb // nch - 1)

    nc = bacc.Bacc("TRN2", target_bir_lowering=False, debug=False)

    xT = nc.dram_tensor("xT", [D_MODEL, BLb], BF16, kind="ExternalInput").ap()
    wq = nc.dram_tensor("wq", [D_MODEL, MLOC], BF16, kind="ExternalInput").ap()
    wk = nc.dram_tensor("wk", [D_MODEL, MLOC], BF16, kind="ExternalInput").ap()
    wv = nc.dram_tensor("wv", [D_MODEL, MLOC], BF16, kind="ExternalInput").ap()
    wo = nc.dram_tensor("wo", [MLOC, D_MODEL], BF16, kind="ExternalInput").ap()
    bq = nc.dram_tensor("bq", [MLOC, 1], F32, kind="ExternalInput").ap()
    bk = nc.dram_tensor("bk", [MLOC, 1], F32, kind="ExternalInput").ap()
    bv = nc.dram_tensor("bv", [MLOC, 1], F32, kind="ExternalInput").ap()
    ident = nc.dram_tensor("ident", [128, 128], BF16, kind="ExternalInput").ap()
    out = nc.dram_tensor("out", [BLb, D_MODEL], BF16, kind="ExternalOutput").ap()

    wqr = wq.rearrange("(k p) m -> p k m", p=128)
    wkr = wk.rearrange("(k p) m -> p k m", p=128)
    wvr = wv.rearrange("(k p) m -> p k m", p=128)

    with tile.TileContext(nc) as tc, ExitStack() as ctx:
        consts = ctx.enter_context(tc.tile_pool(name="consts", bufs=1))
        qk_sb = ctx.enter_context(tc.tile_pool(name="qk_sb", bufs=1))
        xt_pool = ctx.enter_context(tc.tile_pool(name="xt", bufs=2 * KT))
        # PSUM (8 banks): scores 2 slots x 2 banks; av 2 tags x 1 bank;
        # aux 2 slots x 1 bank.
        big_ps = ctx.enter_context(tc.tile_pool(name="big_ps", bufs=2, space="PSUM"))
        av_ps = ctx.enter_context(tc.tile_pool(name="av_ps", bufs=1, space="PSUM"))
        aux_ps = ctx.enter_context(tc.tile_pool(name="aux_ps", bufs=2, space="PSUM"))
        exp_pool = ctx.enter_context(tc.tile_pool(name="expT", bufs=6))
        att_sb = ctx.enter_context(tc.tile_pool(name="att_sb", bufs=2))
        d_pool = ctx.enter_context(tc.tile_pool(name="d_sb", bufs=HPC * 4))
        out_pool = ctx.enter_context(tc.tile_pool(name="out_sb", bufs=6))

        # ---- startup: priority DMA ordering -------------------------------
        wq_sb = consts.tile([128, KT, MLOC], BF16, tag="wq")
        wk_sb = consts.tile([128, KT, MLOC], BF16, tag="wk")
        wv_sb = consts.tile([128, KT, MLOC], BF16, tag="wv")
        wo_sb = consts.tile([128, D_MODEL], BF16, tag="wo")
        bq_sb = consts.tile([MLOC, 1], F32, tag="bq")
        bk_sb = consts.tile([MLOC, 1], F32, tag="bk")
        bv_sb = consts.tile([MLOC, 1], F32, tag="bv")
        id_sb = consts.tile([128, 128], BF16, tag="ident")

        loaded_xts = {}

        def load_xts_for(nc_i, eng):
            csl = slice(nc_i * nch, (nc_i + 1) * nch)
            xts = []
            for k in range(KT):
                xt = xt_pool.tile([128, nch], BF16, tag="xt", name="xt")
                eng.dma_start(xt[:], xT[128 * k : 128 * (k + 1), csl])
                xts.append(xt)
            loaded_xts[nc_i] = xts

        # chunk 0 x tiles + first q/k weight tiles, interleaved by priority
        xts0 = []
        for k in range(KT):
            xt = xt_pool.tile([128, nch], BF16, tag="xt", name="xt")
            xts0.append(xt)
        # Early dummy exp pulls ACT_TABLE_LOAD out of the attention phase.
        ones_bf = consts.tile([1, 128], BF16, tag="ones_bf")
        nc.vector.memset(ones_bf[:], 1.0)
        warm = consts.tile([1, 8], BF16, tag="actwarm")
        nc.scalar.activation(warm[:], ones_bf[0:1, 0:8],
                             mybir.ActivationFunctionType.Exp)

        nc.sync.dma_start(xts0[0][:], xT[0:128, 0:nch])
        nc.sync.dma_start(wq_sb[:, 0, :], wqr[:, 0, :])
        nc.sync.dma_start(wk_sb[:, 0, :], wkr[:, 0, :])
        nc.sync.dma_start(xts0[1][:], xT[128:256, 0:nch])
        nc.sync.dma_start(wq_sb[:, 1, :], wqr[:, 1, :])
        nc.sync.dma_start(wk_sb[:, 1, :], wkr[:, 1, :])
        nc.sync.dma_start(xts0[2][:], xT[256:384, 0:nch])
        # scalar queue is free until the first exp (~25us): bulk x tiles
        for k in range(3, KT):
            nc.scalar.dma_start(xts0[k][:], xT[128 * k : 128 * (k + 1), 0:nch])
        loaded_xts[0] = xts0
        if n_nch > 1:
            load_xts_for(1, nc.scalar)
        for k in range(2, KT):
            nc.gpsimd.dma_start(wq_sb[:, k, :], wqr[:, k, :])
            nc.gpsimd.dma_start(wk_sb[:, k, :], wkr[:, k, :])
        nc.gpsimd.dma_start(bq_sb[:], bq)
        nc.gpsimd.dma_start(bk_sb[:], bk)
        for k in range(KT):
            nc.gpsimd.dma_start(wv_sb[:, k, :], wvr[:, k, :])
        nc.gpsimd.dma_start(bv_sb[:], bv)
        nc.gpsimd.dma_start(wo_sb[:], wo)
        nc.gpsimd.dma_start(id_sb[:], ident)

        # Persistent activations.
        qT_sb = qk_sb.tile([128, BLb], BF16, tag="qT")  # [d_local, b*Lb+l]
        kT_sb = qk_sb.tile([128, BLb], BF16, tag="kT")
        # v (natural layout) + ones column: per (b, head): [128, n_st, 65]
        vaug = [
            [qk_sb.tile([128, n_st, HEAD_DIM + 1], BF16, tag=f"vaug{bi}{h}",
                        name=f"vaug{bi}{h}")
             for h in range(HPC)]
            for bi in range(B)
        ]
        for bi in range(B):
            for h in range(HPC):
                nc.vector.memset(vaug[bi][h][:, :, HEAD_DIM:], 1.0)

        # ---- projection chunk: group-atomic closure list ------------------
        # q, k, v accumulate sequentially, each holding a single 1-bank aux
        # slot; no other aux allocation may interleave within the group.
        def proj_slices(nc_i, prefetch):
            csl = slice(nc_i * nch, (nc_i + 1) * nch)
            cell = {}

            def pq_slice(j):
                def f():
                    if j == 0:
                        cell["ps"] = aux_ps.tile([128, nch], F32, tag="aux",
                                                 name="ps_q")
                    ps, xts = cell["ps"], loaded_xts[nc_i]
                    for k in (2 * j, 2 * j + 1):
                        nc.tensor.matmul(ps[:], wq_sb[:, k, :], xts[k][:],
                                         start=(k == 0), stop=(k == KT - 1))
                    if j == KT // 2 - 1:
                        nc.vector.tensor_scalar_add(qT_sb[:, csl], ps[:],
                                                    bq_sb[:])
                return f

            def pk_slice(j):
                def f():
                    if j == 0:
                        cell["ps"] = aux_ps.tile([128, nch], F32, tag="aux",
                                                 name="ps_k")
                    ps, xts = cell["ps"], loaded_xts[nc_i]
                    for k in (2 * j, 2 * j + 1):
                        nc.tensor.matmul(ps[:], wk_sb[:, k, :], xts[k][:],
                                         start=(k == 0), stop=(k == KT - 1))
                    if j == KT // 2 - 1:
                        nc.vector.tensor_scalar_add(kT_sb[:, csl], ps[:],
                                                    bk_sb[:])
                return f

            def v_slice(stg):
                def f():
                    if stg == 0:
                        cell["psv"] = aux_ps.tile([128, nch], F32, tag="aux",
                                                  name="ps_v")
                    ps_v, xts = cell["psv"], loaded_xts[nc_i]
                    ssl = slice(128 * stg, 128 * (stg + 1))
                    for k in range(KT):
                        nc.tensor.matmul(ps_v[:, ssl], xts[k][:, ssl],
                                         wv_sb[:, k, :],
                                         start=(k == 0), stop=(k == KT - 1))
                    if stg < st_per_nch - 1:
                        return
                    # Batched evacuation: one strided copy per head covers
                    # all s-groups of the chunk.
                    st0 = nc_i * st_per_nch
                    bi, st_b = divmod(st0, n_st)
                    psr = ps_v.rearrange("p (g c) -> p g c", g=st_per_nch)
                    for h in range(HPC):
                        nc.vector.tensor_copy(
                            vaug[bi][h][:, st_b : st_b + st_per_nch, :HEAD_DIM],
                            psr[:, :, HEAD_DIM * h : HEAD_DIM * (h + 1)])
                    if prefetch is not None:
                        load_xts_for(prefetch, nc.sync)
                return f

            return ([pq_slice(j) for j in range(KT // 2)]
                    + [pk_slice(j) for j in range(KT // 2)]
                    + [v_slice(g) for g in range(st_per_nch)])

        # ---- o-path for a finished chunk ----------------------------------
        # av psum [128 l, n_lt, 65] per head; col 64 = denominator per l.
        def opath_dmm_all(o_sb, rcp_sb, cell):
            """All l-tiles' normalize+transpose diagonal matmuls in one
            closure: one column-tiling mode episode for the whole chunk.
            The two heads' D-matmuls run concurrently on column tiles.
            Uses both aux slots (self-contained: allocate, matmul, evac)."""
            def f():
                ds = [[None] * n_lt for _ in range(HPC)]
                for lt in range(n_lt):
                    for h in range(HPC):
                        dt_ = d_pool.tile([128, 128], BF16, tag="D", name="D")
                        nc.vector.tensor_scalar_mul(dt_[:], id_sb[:],
                                                    rcp_sb[:, h, lt : lt + 1])
                        ds[h][lt] = dt_
                for half in range(n_lt // 2):
                    ps_oT = aux_ps.tile([128, 2, 128], F32, tag="aux",
                                        name="ps_oT")
                    for j in range(2):
                        lt = 2 * half + j
                        for h in range(HPC):
                            nc.tensor.matmul(
                                ps_oT[HEAD_DIM * h : HEAD_DIM * (h + 1), j, :],
                                o_sb[:, lt, h, :HEAD_DIM], ds[h][lt][:],
                                start=True, stop=True)
                    for j in range(2):
                        oT = att_sb.tile([128, 128], BF16, tag="oT", name="oT",
                                         bufs=2 * n_lt)
                        nc.vector.tensor_scalar_add(oT[:], ps_oT[:, j, :],
                                                    bv_sb[:])
                        cell[2 * half + j] = oT
            return f

        def oproj_slice(cell, lt, bi, loff, tail=False):
            """Both halves of the o-projection for one l-tile. In the tail
            (after the last exp) ScalarE is free: alternate the evacuation
            engine; out-DMAs rotate between the sync and gpsimd queues."""
            def f():
                oT = cell[lt]
                r0 = bi * Lb + loff + 128 * lt
                for dh in range(2):
                    ps_o = aux_ps.tile([128, 512], F32, tag="aux", name="ps_o")
                    nc.tensor.matmul(ps_o[:], oT[:],
                                     wo_sb[:, 512 * dh : 512 * (dh + 1)],
                                     start=True, stop=True)
                    ob = out_pool.tile([128, 512], BF16, tag="ob")
                    if tail and dh == 0:
                        nc.scalar.activation(ob[:], ps_o[:],
                                             mybir.ActivationFunctionType.Copy)
                    else:
                        nc.vector.tensor_copy(ob[:], ps_o[:])
                    eng = nc.sync if dh == 0 else nc.gpsimd
                    eng.dma_start(
                        out[r0 : r0 + 128, 512 * dh : 512 * (dh + 1)], ob[:])
            return f

        # ---- upfront: batch-0 projections, emitted densely ----------------
        def prefetch_of(nc_i):
            return nc_i + 2 if nc_i + 2 < n_nch else None

        for nc_i in range(upfront):
            for f in proj_slices(nc_i, prefetch_of(nc_i)):
                f()

        # ---- filler scheduling --------------------------------------------
        # proj_groups: group-atomic chains (one closure per period, no
        # interleaving once started). opath_q: ready-gated self-contained
        # singles with priority between groups.
        VTS_BASE = 32000.0  # ~virtual ns at attention start
        VTS_PERIOD = 1160.0  # ~virtual ns per s-tile period (exp-paced)

        def vts(p):
            return (VTS_BASE + VTS_PERIOD * p) / 1e6  # ms for tile_wait_until

        proj_groups = deque()
        period = [0]
        for nc_i in range(upfront, n_nch):
            proj_groups.append(deque(proj_slices(nc_i, prefetch_of(nc_i))))
        opath_q = deque()
        cur_group = [None]

        def pop_fillers(n, force=False):
            for _ in range(n):
                if cur_group[0]:
                    cur_group[0].popleft()()
                    if not cur_group[0]:
                        cur_group[0] = None
                elif opath_q and (opath_q[0][0] <= period[0] or force):
                    opath_q.popleft()[1]()
                elif proj_groups:
                    cur_group[0] = proj_groups.popleft()
                    cur_group[0].popleft()()
                    if not cur_group[0]:
                        cur_group[0] = None
                else:
                    return

        # ---- attention chunks ---------------------------------------------
        chunks = []
        for bi in range(B):
            for lc in range(n_lc):
                chunks.append((bi, lc * lc_size, lc_size))

        def emit_opath(prev, base_ready, tail=False):
            """Evacuate the finished chunk's av psum, compute rcp, and queue
            the per-l-tile oT/o-proj closures."""
            o_sb = att_sb.tile([128, n_lt, HPC, HEAD_DIM + 1], BF16,
                               tag="o_sb", name="o_sb")
            rcp_sb = att_sb.tile([128, HPC, n_lt], F32, tag="rcp", name="rcp")
            for h in range(HPC):
                nc.vector.tensor_copy(o_sb[:, :, h, :], prev["ps"][h][:, :, :])
                nc.vector.reciprocal_approx_fast(
                    rcp_sb[:, h, :], prev["ps"][h][:, :, HEAD_DIM])
            cell = {}
            bi, loff = prev["bi"], prev["loff"]
            opath_q.append((base_ready, opath_dmm_all(o_sb, rcp_sb, cell)))
            for lt in range(n_lt):
                opath_q.append((base_ready + 2 + lt,
                                oproj_slice(cell, lt, bi, loff, tail=tail)))

        prev = None  # previous chunk's state; its last avs are emitted here
        for ci, (bi, loff, width) in enumerate(chunks):
            lsl = slice(bi * Lb + loff, bi * Lb + loff + width)
            cellav = {}
            exs = [None] * n_st

            def do_sc(st):
                ssl = slice(bi * Lb + st * 128, bi * Lb + (st + 1) * 128)
                ps_sc = big_ps.tile([128, HPC, lc_size], F32, tag="big",
                                    name="ps_sc")
                for h in range(HPC):
                    hsl = slice(64 * h, 64 * (h + 1))
                    nc.tensor.matmul(ps_sc[:, h, :width], kT_sb[hsl, ssl],
                                     qT_sb[hsl, lsl],
                                     start=True, stop=True,
                                     tile_position=(64 * h, 0))
                ex = exp_pool.tile([128, HPC, lc_size], BF16, tag="ex",
                                   name="ex")
                nc.scalar.activation(ex[:, :, :width], ps_sc[:, :, :width],
                                     mybir.ActivationFunctionType.Exp,
                                     scale=1.0 / np.sqrt(HEAD_DIM))
                exs[st] = ex

            def do_av(st, bi=bi, exs=exs, cellav=cellav):
                # One accumulation group per PSUM bank (per head): start
                # zero-marks the whole 2KB region, so the first touch of
                # every lt sub-region overwrites; only (st=0, lt=0) starts
                # and only the final write stops.
                for h in range(HPC):
                    for lt in range(n_lt):
                        nc.tensor.matmul(
                            cellav["ps"][h][:, lt, :],
                            exs[st][:, h, 128 * lt : 128 * (lt + 1)],
                            vaug[bi][h][:, st, :],
                            start=(st == 0 and lt == 0),
                            stop=(st == n_st - 1 and lt == n_lt - 1))

            lag = 2 if n_st > 4 else 1
            # Previous chunk's last `lag` avs + its o-path run interleaved
            # with this chunk's first scores. Each period is stamped with a
            # virtual-time floor (tile_wait_until) so the Tile scheduler's
            # compile-time linearization cannot front-load filler work and
            # starve the exp pipeline.
            for st in range(lag):
                with tc.tile_wait_until(vts(period[0])):
                    do_sc(st)
                    if prev is not None:
                        prev["do_av"](n_st - lag + st)
                        if st == lag - 1:
                            emit_opath(prev, period[0] + 2)
                            prev = None
                    pop_fillers(1)
                period[0] += 1
            cellav["ps"] = [av_ps.tile([128, n_lt, HEAD_DIM + 1], F32,
                                       tag=f"av{h}", name=f"av{h}")
                            for h in range(HPC)]
            for st in range(lag, n_st):
                with tc.tile_wait_until(vts(period[0])):
                    do_sc(st)
                    do_av(st - lag)
                    pop_fillers(1)
                period[0] += 1
            prev = {"do_av": do_av, "ps": cellav["ps"], "bi": bi,
                    "loff": loff, "width": width}

        # ---- tail: last chunk's o-path -------------------------------------
        with tc.tile_wait_until(vts(period[0])):
            for st in range(n_st - lag, n_st):
                pop_fillers(2, force=True)
                prev["do_av"](st)
            emit_opath(prev, period[0], tail=True)
            while cur_group[0] or opath_q or proj_groups:
                pop_fillers(1, force=True)

    nc.compile()
    return nc


def make_in_maps(x, Wq, bq, Wk, bk, Wv, bv, Wo, Lb=L):
    """Per-core input dicts from full inputs."""
    BLb = B * Lb
    xT = np.ascontiguousarray(
        np.asarray(x, np.float32).reshape(BLb, D_MODEL).T).astype(NPBF16)
    Wq = np.asarray(Wq, np.float32).astype(NPBF16)
    Wk = np.asarray(Wk, np.float32).astype(NPBF16)
    Wv = np.asarray(Wv, np.float32).astype(NPBF16)
    Wo = np.asarray(Wo, np.float32).astype(NPBF16)
    ident = np.eye(128, dtype=NPBF16)
    in_maps = []
    for c in range(N_CORES):
        dsl = slice(MLOC * c, MLOC * (c + 1))
        in_maps.append({
            "xT": xT,
            "wq": np.ascontiguousarray(Wq[:, dsl]),
            "wk": np.ascontiguousarray(Wk[:, dsl]),
            "wv": np.ascontiguousarray(Wv[:, dsl]),
            "wo": np.ascontiguousarray(Wo[dsl, :]),
            "bq": np.ascontiguousarray(np.asarray(bq, np.float32)[dsl].reshape(MLOC, 1)),
            "bk": np.ascontiguousarray(np.asarray(bk, np.float32)[dsl].reshape(MLOC, 1)),
            "bv": np.ascontiguousarray(np.asarray(bv, np.float32)[dsl].reshape(MLOC, 1)),
            "ident": ident,
        })
    return in_maps


_NC_CACHE = {}


def _get_nc():
    if "nc" not in _NC_CACHE:
        _NC_CACHE["nc"] = build_nc()
    return _NC_CACHE["nc"]


def kernel(x, Wq, bq, Wk, bk, Wv, bv, Wo, bo):
    nc = _get_nc()
    in_maps = make_in_maps(x, Wq, bq, Wk, bk, Wv, bv, Wo)
    res = run_bass_kernel_spmd(nc, in_maps, list(range(N_CORES)))
    acc = np.zeros((B * L, D_MODEL), dtype=np.float32)
    for c in range(N_CORES):
        acc += np.asarray(res.results[c]["out"], dtype=np.float32)
    acc += np.asarray(bo, dtype=np.float32)
    return acc.reshape(B, L, D_MODEL)
